# revision 37
# baseline (speedup 1.0000x reference)
"""Trainium2 Bass kernel for nn_DualAxisAggAttn (dual-axis aggregation attention).

Reference semantics per batch image x[C=256, H=64, W=64], twice (W axis then H axis):
  qkv = conv1x1(x) -> {q:[1], k:[C], v:[C]};  s = softmax_axis(q)
  ctx[c,a] = sum_r k*s;  out = x + sigmoid(v) * ctx_bcast;  y = conv1x1(out)

Distribution: data-parallel over batch (16 images -> 2 per NeuronCore x 8 cores).

Key optimizations:
  - key-path linearity: ctx = Wk @ (sum_r x*E) / S -- the key 1x1 conv moves
    AFTER the softmax-weighted reduction (N=4096 -> N=64 moving columns).
  - combine folded into the fusion matmul: ps = Wf@x + Wf@g2 (psum accum),
    so `out = x + g2` is never materialized.
  - query row replicated 128x in its m-tile -> exp(q) lands partition-broadcast.
  - sigmoid via tanh ((1+tanh(v/2))/2): exp+tanh+copy share ONE ACT table set;
    the +1 is applied in-place on the gate (4x tensor_scalar), the 0.5 folds
    into the softmax normalizer and k-bias.
  - all matmuls bf16 (host pre-casts inputs; measured rel err ~3e-3 vs fp32).
  - reductions inner-contiguous (stage W halve+reduce, stage H contiguous
    binary tree over h); no strided elementwise ops.
  - per-engine instruction streams are FIFO, so batch-stage PHASES are
    interleaved at emission time to keep the PE fed during reduce chains.
  - GPSIMD does nothing (it contends with DVE for the shared SBUF port).
"""

import numpy as np
import ml_dtypes
from contextlib import ExitStack

import concourse.bass as bass
import concourse.bacc as bacc
import concourse.tile as tile
import concourse.mybir as mybir
from concourse.bass_utils import run_bass_kernel_spmd

F32 = mybir.dt.float32
BF16 = mybir.dt.bfloat16
AF = mybir.ActivationFunctionType
ALU = mybir.AluOpType
AX = mybir.AxisListType
NPBF = ml_dtypes.bfloat16

B, C, H, W = 16, 256, 64, 64
HW = H * W
NCORES = 8
BPC = B // NCORES
KT = 2
CH = 512
NCH = HW // CH
GRP = CH // 64

_BUILD_CACHE = {}
LAST_RESULTS = None


class _Stage:
    """Emits one attention stage (one batch) in three phases."""

    def __init__(self, nc, pools, axis_w, srcs, stat, wk, fus, bias, dst_evict):
        self.nc, self.axis_w = nc, axis_w
        self.phv = pools[-1]
        self.srcs, self.stat, self.wk, self.fus = srcs, stat, wk, fus
        self.bias, self.dst_evict = bias, dst_evict
        (self.pbig, self.pgate, self.pchunk, self.pctx, self.pq, self.pv, self.pf) = pools[:7]

    def p1_alloc(self):
        self.E = self.pbig.tile([128, HW], BF16, tag="E")
        self.gate = self.pgate.tile([128, 2, HW], BF16, tag="gate")
        self.u0 = self.pbig.tile([128, HW], BF16, tag="u0")
        self.u1 = self.pbig.tile([128, HW], BF16, tag="u1")

    def p1_chunk(self, j):
        nc, srcs, stat, bias = self.nc, self.srcs, self.stat, self.bias
        E, gate, u0, u1 = self.E, self.gate, self.u0, self.u1
        bv2 = bias.get("bv2")
        if True:
            sl = bass.ts(j, CH)
            ps_q = self.pq.tile([128, CH], F32, tag="q")
            ps_v = self.pv.tile([128, 2 * CH], F32, tag="vf")
            for kt in range(KT):
                st, sp = kt == 0, kt == KT - 1
                rhs = srcs[j][:, kt, :]
                nc.tensor.matmul(ps_q[:], stat[:, kt, 2, :], rhs, start=st, stop=sp)
                nc.tensor.matmul(ps_v[:, 0:CH], stat[:, kt, 0, :], rhs, start=st, stop=sp)
                nc.tensor.matmul(ps_v[:, CH:], stat[:, kt, 1, :], rhs, start=st, stop=sp)
            nc.scalar.activation(E[:, sl], ps_q[:], AF.Exp, bias=bias["zb"])
            if bv2 is None:
                nc.scalar.activation(
                    gate[:, :, sl], ps_v[:].rearrange("p (c n) -> p c n", c=2),
                    AF.Tanh, bias=bias["zb"], scale=0.5,
                )
            else:
                nc.scalar.activation(gate[:, 0, sl], ps_v[:, 0:CH], AF.Tanh, bias=bv2[0], scale=0.5)
                nc.scalar.activation(gate[:, 1, sl], ps_v[:, CH:], AF.Tanh, bias=bv2[1], scale=0.5)
            nc.vector.tensor_tensor(u0[:, sl], srcs[j][:, 0, :], E[:, sl], op=ALU.mult)
            nc.vector.tensor_tensor(u1[:, sl], srcs[j][:, 1, :], E[:, sl], op=ALU.mult)
            if not self.axis_w:
                nc.vector.tensor_scalar_add(gate[:, :, sl], gate[:, :, sl], 1.0)

    def _reduce64(self, flat, tag):
        nc, pctx = self.nc, self.pctx
        if self.axis_w:
            v3 = flat.rearrange("p (a r) -> p a r", r=64)
            hv = self.phv.tile([128, 64, 32], BF16, tag="hv")
            nc.vector.tensor_tensor(hv[:], v3[:, :, 0:32], v3[:, :, 32:64], op=ALU.add)
            h2 = self.phv.tile([128, 64, 16], BF16, tag="hv2")
            nc.vector.tensor_tensor(h2[:], hv[:, :, 0:16], hv[:, :, 16:32], op=ALU.add)
            out = pctx.tile([128, 64], F32, tag=f"red_{tag}")
            nc.vector.tensor_reduce(out[:], h2[:], axis=AX.X, op=ALU.add)
        else:
            t = self.phv.tile([128, 2048], BF16, tag="tree")
            nc.vector.tensor_tensor(t[:], flat[:, 0:2048], flat[:, 2048:4096], op=ALU.add)
            n = 1024
            while n >= 128:
                nc.vector.tensor_tensor(t[:, 0:n], t[:, 0:n], t[:, n : 2 * n], op=ALU.add)
                n //= 2
            out = pctx.tile([128, 64], F32, tag=f"red_{tag}")
            nc.vector.tensor_tensor(out[:], t[:, 0:64], t[:, 64:128], op=ALU.add)
        return out

    def p2(self):
        nc, pctx, bias = self.nc, self.pctx, self.bias
        S = self._reduce64(self.E[:], "S")
        R = pctx.tile([128, 64], F32, tag="R")
        nc.vector.reciprocal(R[:], S[:])
        xen = []
        for ct, u in enumerate((self.u0, self.u1)):
            xe = self._reduce64(u[:], f"xe{ct}")
            xn = pctx.tile([128, 64], BF16, tag=f"xn{ct}")
            nc.vector.tensor_tensor(xn[:], xe[:], R[:], op=ALU.mult)
            xen.append(xn)
        self.ctxs = []
        bk2 = bias.get("bk2")
        for mt in range(2):
            ps_c = self.pq.tile([128, 64], F32, tag="q")
            for ct in range(2):
                nc.tensor.matmul(ps_c[:], self.wk[:, ct, mt, :], xen[ct][:], start=ct == 0, stop=ct == 1)
            cn = pctx.tile([128, 64], BF16, tag=f"cn{mt}")
            if bk2 is None:
                nc.vector.tensor_scalar_mul(cn[:], ps_c[:], 0.5)
            else:
                nc.vector.tensor_scalar(cn[:], ps_c[:], 0.5, bk2[mt], op0=ALU.mult, op1=ALU.add)
            self.ctxs.append(cn)

    def p3_chunk(self, j):
        nc, srcs, fus = self.nc, self.srcs, self.fus
        if True:
            sl = bass.ts(j, CH)
            g2s = []
            for ct in range(2):
                if self.axis_w:
                    cb = self.ctxs[ct][:, bass.ts(j, GRP)].unsqueeze(2).broadcast_to([128, GRP, 64])
                else:
                    cb = self.ctxs[ct][:].unsqueeze(1).broadcast_to([128, GRP, 64])
                g2 = self.pchunk.tile([128, GRP, 64], BF16, tag=f"g2_{ct}")
                gv = self.gate[:, ct, sl].rearrange("p (a r) -> p a r", r=64)
                if self.axis_w:
                    nc.vector.scalar_tensor_tensor(g2[:], gv, 1.0, cb, op0=ALU.add, op1=ALU.mult)
                else:
                    nc.vector.tensor_tensor(g2[:], gv, cb, op=ALU.mult)
                g2s.append(g2)
            ps_f = self.pf.tile([128, 2 * CH], F32, tag="vf")
            for mt in range(2):
                half = ps_f[:, bass.ts(mt, CH)]
                nc.tensor.matmul(half, fus[:, 0, mt, :], srcs[j][:, 0, :], start=True, stop=False)
                nc.tensor.matmul(half, fus[:, 1, mt, :], srcs[j][:, 1, :], start=False, stop=False)
                nc.tensor.matmul(half, fus[:, 0, mt, :], g2s[0][:].rearrange("p a r -> p (a r)"), start=False, stop=False)
                nc.tensor.matmul(half, fus[:, 1, mt, :], g2s[1][:].rearrange("p a r -> p (a r)"), start=False, stop=True)
            self.dst_evict(j, ps_f)


def _build(flags):
    bvW0, bkW0, bvH0, bkH0, bfW0, bfH0 = flags
    nc = bacc.Bacc(trn_type="TRN2", target_bir_lowering=False, debug=False)

    x_d = nc.dram_tensor("x", [BPC, C, HW], BF16, kind="ExternalInput").ap()
    statW_d = nc.dram_tensor("statW", [128, KT, 3, 128], BF16, kind="ExternalInput").ap()
    statH_d = nc.dram_tensor("statH", [128, KT, 3, 128], BF16, kind="ExternalInput").ap()
    wkW_d = nc.dram_tensor("wkW", [128, KT, 2, 128], BF16, kind="ExternalInput").ap()
    wkH_d = nc.dram_tensor("wkH", [128, KT, 2, 128], BF16, kind="ExternalInput").ap()
    fusW_d = nc.dram_tensor("fusW", [128, KT, 2, 128], BF16, kind="ExternalInput").ap()
    fusH_d = nc.dram_tensor("fusH", [128, KT, 2, 128], BF16, kind="ExternalInput").ap()
    bias_d = nc.dram_tensor("biases", [6, 2, 128], F32, kind="ExternalInput").ap()
    y_d = nc.dram_tensor("y", [BPC, C, HW], F32, kind="ExternalOutput").ap()

    with tile.TileContext(nc) as tc, ExitStack() as ctx:
        wp = ctx.enter_context(tc.tile_pool(name="weights", bufs=1))
        xbp = ctx.enter_context(tc.tile_pool(name="xbf", bufs=18))
        xwp = ctx.enter_context(tc.tile_pool(name="xw", bufs=16))
        pbig = ctx.enter_context(tc.tile_pool(name="big", bufs=2))
        pgate = ctx.enter_context(tc.tile_pool(name="gate", bufs=2))
        pchunk = ctx.enter_context(tc.tile_pool(name="chunk", bufs=3))
        pctx = ctx.enter_context(tc.tile_pool(name="ctx", bufs=3))
        phv = ctx.enter_context(tc.tile_pool(name="hv", bufs=2))
        yp = ctx.enter_context(tc.tile_pool(name="yev", bufs=3))
        pq = ctx.enter_context(tc.tile_pool(name="psq", bufs=2, space="PSUM"))
        pvf = ctx.enter_context(tc.tile_pool(name="psvf", bufs=3, space="PSUM"))
        pools = (pbig, pgate, pchunk, pctx, pq, pvf, pvf, phv)

        def wload(name, dram, shape, dt):
            t = wp.tile(shape, dt, tag=name)
            nc.scalar.dma_start(t[:], dram[:])
            return t

        statW = wload("statW", statW_d, [128, KT, 3, 128], BF16)
        statH = wload("statH", statH_d, [128, KT, 3, 128], BF16)
        wkW = wload("wkW", wkW_d, [128, KT, 2, 128], BF16)
        wkH = wload("wkH", wkH_d, [128, KT, 2, 128], BF16)
        fusW = wload("fusW", fusW_d, [128, KT, 2, 128], BF16)
        fusH = wload("fusH", fusH_d, [128, KT, 2, 128], BF16)

        bias_sb = wp.tile([128, 6, 2], F32, tag="biases")
        nc.scalar.dma_start(bias_sb[:], bias_d[:].transpose([2, 0, 1]))
        zb = wp.tile([128, 1], F32, tag="zb")
        nc.vector.memset(zb[:], 0.0)

        def bap(i, ct):
            return bias_sb[:, i, ct].unsqueeze(1)

        biasW = {
            "bv2": None if bvW0 else [bap(0, ct) for ct in range(2)],
            "bk2": None if bkW0 else [bap(1, ct) for ct in range(2)],
            "zb": zb[:],
        }
        biasH = {
            "bv2": None if bvH0 else [bap(2, ct) for ct in range(2)],
            "bk2": None if bkH0 else [bap(3, ct) for ct in range(2)],
            "zb": zb[:],
        }

        def load_x(b):
            xcs = []
            for j in range(NCH):
                xc = xbp.tile([128, KT, CH], BF16, tag="xc")
                for kt in range(KT):
                    nc.sync.dma_start(xc[:, kt, :], x_d[b, bass.ts(kt, 128), bass.ts(j, CH)])
                xcs.append(xc[:])
            return xcs

        def make_W(b, xcs):
            xw_tiles = [None] * NCH

            def evW(j, ps_f):
                xw = xwp.tile([128, KT, CH], BF16, tag="xw")
                xw_tiles[j] = xw[:]
                if bfW0:
                    nc.scalar.activation(xw[:], ps_f[:].rearrange("p (c n) -> p c n", c=2), AF.Copy)
                else:
                    for ct in range(2):
                        nc.scalar.activation(
                            xw[:, ct, :], ps_f[:, bass.ts(ct, CH)],
                            AF.Identity, bias=bap(4, ct),
                        )

            st = _Stage(nc, pools, True, xcs, statW, wkW, fusW, biasW, evW)
            st.xw_tiles = xw_tiles
            return st

        def make_H(b, xw_tiles):
            def evH(j, ps_f):
                y_t = yp.tile([128, 2, CH], F32, tag="y")
                if bfH0:
                    nc.scalar.activation(y_t[:], ps_f[:].rearrange("p (c n) -> p c n", c=2), AF.Copy)
                else:
                    for ct in range(2):
                        nc.scalar.activation(
                            y_t[:, ct, :], ps_f[:, bass.ts(ct, CH)],
                            AF.Identity, bias=bap(5, ct),
                        )
                nc.sync.dma_start(
                    y_d[b].rearrange("(c p) n -> p c n", p=128)[:, :, bass.ts(j, CH)],
                    y_t[:],
                )

            return _Stage(nc, pools, False, xw_tiles, statH, wkH, fusH, biasH, evH)

        # interleaved phase schedule: chunk-level alternation keeps every
        # engine's FIFO stream fed during the other phase's stalls
        def run_p1(st):
            st.p1_alloc()
            for j in range(NCH):
                st.p1_chunk(j)

        def run_p3(st):
            for j in range(NCH):
                st.p3_chunk(j)

        x0 = load_x(0)
        x1 = load_x(1)
        w0 = make_W(0, x0)
        w1 = make_W(1, x1)
        run_p1(w0)
        run_p1(w1)
        w0.p2()
        run_p3(w0)
        h0 = make_H(0, w0.xw_tiles)
        w1.p2()
        run_p1(h0)
        run_p3(w1)
        h1 = make_H(1, w1.xw_tiles)
        h0.p2()
        run_p1(h1)
        run_p3(h0)
        h1.p2()
        run_p3(h1)

    nc.compile()
    return nc


def _prep(qkv_w, fus_w):
    wq = qkv_w[0]
    wk = qkv_w[1 : 1 + C]
    wv = qkv_w[1 + C :]
    stat = np.empty((128, KT, 3, 128), np.float32)
    wkt = np.empty((128, KT, 2, 128), np.float32)
    fus = np.empty((128, KT, 2, 128), np.float32)
    for kt in range(KT):
        cs = slice(kt * 128, (kt + 1) * 128)
        stat[:, kt, 0, :] = wv[0:128, cs].T
        stat[:, kt, 1, :] = wv[128:256, cs].T
        stat[:, kt, 2, :] = np.repeat(wq[cs][:, None], 128, axis=1)
        wkt[:, kt, 0, :] = wk[0:128, cs].T
        wkt[:, kt, 1, :] = wk[128:256, cs].T
        fus[:, kt, 0, :] = fus_w[0:128, cs].T
        fus[:, kt, 1, :] = fus_w[128:256, cs].T
    tobf = lambda a: np.ascontiguousarray(a.astype(NPBF))
    return tobf(stat), tobf(wkt), tobf(fus)


def kernel(x, qkvW_w, qkvW_b, qkvH_w, qkvH_b, fusW_w, fusW_b, fusH_w, fusH_b):
    global LAST_RESULTS
    x = np.asarray(x, np.float32)
    qkvW_w = np.asarray(qkvW_w, np.float32)
    qkvW_b = np.asarray(qkvW_b, np.float32)
    qkvH_w = np.asarray(qkvH_w, np.float32)
    qkvH_b = np.asarray(qkvH_b, np.float32)
    fusW_w = np.asarray(fusW_w, np.float32)
    fusW_b = np.asarray(fusW_b, np.float32)
    fusH_w = np.asarray(fusH_w, np.float32)
    fusH_b = np.asarray(fusH_b, np.float32)

    statW, wkW, fusW = _prep(qkvW_w, fusW_w)
    statH, wkH, fusH = _prep(qkvH_w, fusH_w)

    bkW = qkvW_b[1 : 1 + C]
    bvW = qkvW_b[1 + C :]
    bkH = qkvH_b[1 : 1 + C]
    bvH = qkvH_b[1 + C :]
    biases = np.stack(
        [0.5 * bvW.reshape(2, 128),
         0.5 * bkW.reshape(2, 128),
         0.5 * bvH.reshape(2, 128),
         0.5 * bkH.reshape(2, 128),
         fusW_b.reshape(2, 128),
         fusH_b.reshape(2, 128)]
    ).astype(np.float32)

    flags = (
        not bvW.any(), not bkW.any(), not bvH.any(), not bkH.any(),
        not fusW_b.any(), not fusH_b.any(),
    )
    if flags not in _BUILD_CACHE:
        _BUILD_CACHE[flags] = _build(flags)
    nc = _BUILD_CACHE[flags]

    xbf = np.ascontiguousarray(x.reshape(B, C, HW).astype(NPBF))
    in_maps = []
    for core in range(NCORES):
        in_maps.append({
            "x": xbf[core * BPC : (core + 1) * BPC],
            "statW": statW, "statH": statH,
            "wkW": wkW, "wkH": wkH, "fusW": fusW, "fusH": fusH,
            "biases": biases,
        })

    res = run_bass_kernel_spmd(nc, in_maps, list(range(NCORES)))
    LAST_RESULTS = res
    y = np.concatenate([r["y"] for r in res.results], axis=0)
    return y.reshape(B, C, H, W)
